# revision 1
# baseline (speedup 1.0000x reference)
"""Trainium2 Bass kernel for degree-3 uniform B-spline basis evaluation.

Problem: x (1024, 8192) fp32, knots = linspace(-2, 2, 12) -> out (1024, 8192, 8)
where out[..., i] is the i-th cubic B-spline basis function (Cox-de Boor).

Math. With uniform knots (spacing h), basis i is a shifted cardinal cubic
B-spline: out_i(x) = C((x - knots[0])/h - i), C supported on [0, 4). Writing
a = |(x - knots[0])/h - i - 2| (distance to the support center), C reflects to

    C = relu(2 - a)^3 / 6  -  (2/3) * relu(1 - a)^3

which is numerically clean (all operands O(1), no cancellation of large
truncated powers) and returns *exact* zeros outside the support, matching the
reference's indicator-based recursion even for |x| beyond the grid.

Kernel (hybrid, balancing VectorE and ScalarE):

Four channels run entirely on VectorE as two custom DVE ops each, reading the
x tile directly (k1 = 6^(-1/3), k2 = (2/3)^(1/3), c_i = knots[0] + (i+2) h):

    BSPL_P: p   = w * sq(relu(w)),        w  = 2*k1 - |x - c_i| * (k1/h)
    BSPL_Q: out = p - w1 * sq(relu(w1)),  w1 = k2   - |x - c_i| * (k2/h)

The other four channels offload the |.|/relu prologue to the otherwise-idle
ScalarE (ACT) as wide-span activation chains (amortizing ACT's per-op
overhead), leaving ONE single-source DVE op per channel:

    ACT:  a  = Abs(x/h - c_i/h);  rs = Relu(k1*(2 - a))      [rs = k1*relu(2-a)]
    DVE:  BSPL_C3: out = sq(rs)*rs - w1*sq(relu(w1)),  w1 = (k2/k1)*rs - k2

(sq(rs)*rs = relu(2-a)^3/6 exactly, and w1 = k2*(1-a) wherever rs > 0; where
rs = 0 the channel is outside its support and w1 = -k2 < 0, so the relu zeroes
the second term too — exact zeros preserved.) Each custom op is a single
<=8-stage DVE instruction at 1 elem/lane/cycle, so DVE issues 12 instructions
per [128, 1024] tile instead of 16, with ACT carrying 8 wide ops per
2048-span. All channel results are written straight into an
interleaved [P, F, 8] SBUF tile (stride-8 free-dim APs) so every store DMA is
one fully contiguous transfer; the il pool is triple-buffered so compute is
not WAR-blocked on store-DMA drain (the ACT-transient `a` pool is single-
buffered to fund it — free, since ACT executes in order).

Sharding: batch-parallel, rows 128*c .. 128*c+127 on core c (8 cores).
"""

import numpy as np

_CACHE = {}

_K1 = float(6.0 ** (-1.0 / 3.0))        # k1^3 = 1/6
_K2 = float((2.0 / 3.0) ** (1.0 / 3.0))  # k2^3 = 2/3
_SQ6 = float(6.0 ** -0.5)

_P = 128          # SBUF partitions = rows per core
_COLS = 8192      # row length
_NB = 8           # basis functions
_F = 1024         # free-dim chunk per DVE tile / store DMA
_FA = 2048        # free-dim span per ACT chain
_N_ACT = 4        # channels offloaded to the ScalarE pipeline
_NCORES = 8


def _register_custom_ops():
    import concourse.dve_ops as dve_ops
    from concourse.dve_ops import DveOp
    from concourse.dve_spec import (
        Spec, Src0, Src1, C0, C1, C2, One, relu, sq, lower, AluOp, Bin,
    )
    from concourse.dve_uop import DveOpSpec

    def ref_p(in0, in1, s0, s1, imm2):
        w = imm2 - np.abs(in0.astype(np.float32) - s0) * s1
        return (np.square(np.maximum(w, 0)) * w).astype(np.float32)

    def ref_q(in0, in1, s0, s1, imm2):
        w = imm2 - np.abs(in0.astype(np.float32) - s0) * s1
        return (in1 - np.square(np.maximum(w, 0)) * w).astype(np.float32)

    def ref_c3(in0, in1, s0, s1, imm2):
        rs = in0.astype(np.float32)
        p = np.square(rs) * rs
        w1 = rs * s0 - s1
        s = np.maximum(w1, 0)
        return (p - np.square(s) * w1).astype(np.float32)

    def body_p():
        w = C2 - Bin(AluOp.ABSOLUTE_DIFF, Src0, C0) * C1
        return sq(relu(w)) * w

    def body_q():
        w1 = C2 - Bin(AluOp.ABSOLUTE_DIFF, Src0, C0) * C1
        return Src1 - sq(relu(w1)) * w1

    def body_c3():
        p = sq(Src0) * Src0
        w1 = Src0 * C0 - C1
        return p - sq(relu(w1)) * w1

    def make(name, body, ref):
        spec = Spec(body=body, reference=ref)
        shas = {}
        for ver in ("v3", "v4"):
            shas[ver] = DveOpSpec(name=name, uops=lower(spec, ver=ver)).sha(ver)
        return DveOp(name, spec, subdim=False, uops_sha=shas)

    ops = {}
    for name, body, ref in (
        ("BSPL_P", body_p(), ref_p),
        ("BSPL_Q", body_q(), ref_q),
        ("BSPL_C3", body_c3(), ref_c3),
    ):
        existing = {op.name: op for op in dve_ops.OPS}
        if name in existing:
            ops[name] = existing[name]
            continue
        op = make(name, body, ref)
        dve_ops.OPS.append(op)
        dve_ops.CUSTOM_DVE_SPECS[op.name] = op.spec
        row = max(dve_ops._SUB_OPCODE_FOR_NAME.values()) + 1
        assert row < 0x20
        dve_ops._SUB_OPCODE_FOR_NAME[op.name] = row
        ops[name] = op
    return ops["BSPL_P"], ops["BSPL_Q"], ops["BSPL_C3"]


def _build(knot0: float, h: float, passes: int = 1):
    import concourse.bacc as bacc
    import concourse.mybir as mybir
    from concourse import tile

    AF = mybir.ActivationFunctionType
    bspl_p, bspl_q, bspl_c3 = _register_custom_ops()

    nc = bacc.Bacc("TRN2", target_bir_lowering=False, debug=False,
                   num_devices=_NCORES)
    x_ext = nc.declare_dram_parameter("x", [_P, _COLS], mybir.dt.float32,
                                      isOutput=False)
    out_ext = nc.declare_dram_parameter("out", [_P, _COLS * _NB],
                                        mybir.dt.float32, isOutput=True)
    act_ch = list(range(_NB - _N_ACT, _NB))

    with tile.TileContext(nc) as tc:
        with tc.tile_pool(name="xin", bufs=2) as xin, \
             tc.tile_pool(name="ilp", bufs=3) as ilp, \
             tc.tile_pool(name="wk", bufs=2) as wk, \
             tc.tile_pool(name="ap", bufs=1) as apool, \
             tc.tile_pool(name="rp", bufs=2) as rpool, \
             tc.tile_pool(name="cst", bufs=1) as cst:
            # ACT's float bias operands must live in SBUF as [P, 1] const APs.
            cvals = sorted({-(knot0 + (i + 2) * h) / h for i in act_ch}
                           | {2.0 * _K1})
            for v in cvals:
                t = cst.tile([_P, 1], mybir.dt.float32, tag=f"c{v}")
                nc.vector.memset(t[:], float(v))
                nc.const_aps.aps[(mybir.dt.float32, float(v))] = t
            for rep in range(passes):
                # Software-pipelined emission: issue span s+1's load + ACT
                # chain before span s's DVE/store work so Tile's priority
                # heap overlaps ACT prologues with the previous span's
                # compute (cost model: 144.5us -> 127.9us).
                def _load_act(s):
                    xs = xin.tile([_P, _FA], mybir.dt.float32, tag="x")
                    for half in range(_FA // _F):
                        nc.sync.dma_start(
                            xs[:, half * _F:(half + 1) * _F],
                            x_ext[:, s * _FA + half * _F:
                                  s * _FA + (half + 1) * _F])
                    rss = {}
                    for i in act_ch:
                        c_i = knot0 + (i + 2) * h
                        a = apool.tile([_P, _FA], mybir.dt.float32, tag="a")
                        nc.scalar.activation(a[:], xs[:], AF.Abs,
                                             bias=-c_i / h, scale=1.0 / h)
                        rs = rpool.tile([_P, _FA], mybir.dt.float32,
                                        tag=f"rs{i}")
                        nc.scalar.activation(rs[:], a[:], AF.Relu,
                                             bias=2.0 * _K1, scale=-_K1)
                        rss[i] = rs
                    return xs, rss
                nspan = _COLS // _FA
                pending = _load_act(0)
                for s in range(nspan):
                    xs, rss = pending
                    if s + 1 < nspan:
                        pending = _load_act(s + 1)
                    for half in range(_FA // _F):
                        c = s * (_FA // _F) + half
                        lo, hi = half * _F, (half + 1) * _F
                        il = ilp.tile([_P, _F, _NB], mybir.dt.float32,
                                      tag="il")
                        for i in range(_NB):
                            c_i = knot0 + (i + 2) * h
                            if i in act_ch:
                                nc.vector._custom_dve(
                                    bspl_c3, out=il[:, :, i],
                                    in0=rss[i][:, lo:hi],
                                    s0=_K2 / _K1, s1=_K2)
                            else:
                                p = wk.tile([_P, _F], mybir.dt.float32,
                                            tag="p")
                                nc.vector._custom_dve(
                                    bspl_p, out=p[:], in0=xs[:, lo:hi],
                                    s0=c_i, s1=_K1 / h, imm2=2.0 * _K1)
                                nc.vector._custom_dve(
                                    bspl_q, out=il[:, :, i],
                                    in0=xs[:, lo:hi], in1=p[:],
                                    s0=c_i, s1=_K2 / h, imm2=_K2)
                        nc.sync.dma_start(
                            out_ext[:, c * _F * _NB:(c + 1) * _F * _NB],
                            il.rearrange("p f e -> p (f e)"))

    nc.compile()
    return nc


def _numpy_fallback(x, knots):
    """Cox-de Boor on host — only used if knots are not uniform (the
    reference always generates uniform knots; this is a safety net)."""
    te = x[..., None]
    B = ((knots[:-1] <= te) & (te < knots[1:])).astype(np.float32)
    nk = len(knots)
    for k in range(1, 4):
        n = nk - k - 1
        ld = knots[k:k + n] - knots[:n]
        rd = knots[k + 1:k + 1 + n] - knots[1:1 + n]
        left = np.where(ld != 0, (te - knots[:n]) / ld, 0.0) * B[..., :n]
        right = (np.where(rd != 0, (knots[k + 1:k + 1 + n] - te) / rd, 0.0)
                 * B[..., 1:n + 1])
        B = (left + right).astype(np.float32)
    return B[..., :_NB]


def kernel(x: np.ndarray, knots: np.ndarray | None = None, **_ignored):
    from concourse.bass_utils import run_bass_kernel_spmd

    x = np.ascontiguousarray(np.asarray(x, dtype=np.float32))
    if knots is None:
        knots = np.linspace(-2.0, 2.0, 12, dtype=np.float32)
    knots = np.asarray(knots, dtype=np.float32)
    assert x.shape == (_P * _NCORES, _COLS), x.shape
    knot0 = float(knots[0])
    h = float(knots[-1] - knots[0]) / (len(knots) - 1)
    if not np.allclose(np.diff(knots), h, rtol=1e-5, atol=1e-6):
        return _numpy_fallback(x, knots)

    key = (knot0, h)
    if key not in _CACHE:
        _CACHE[key] = _build(knot0, h)
    nc = _CACHE[key]

    in_maps = [{"x": x[c * _P:(c + 1) * _P]} for c in range(_NCORES)]
    res = run_bass_kernel_spmd(nc, in_maps, list(range(_NCORES)))
    out = np.empty((_P * _NCORES, _COLS, _NB), dtype=np.float32)
    for c in range(_NCORES):
        out[c * _P:(c + 1) * _P] = res.results[c]["out"].reshape(_P, _COLS, _NB)
    return out

